# revision 18
# baseline (speedup 1.0000x reference)
"""Trainium2 Bass/Tile kernel for per-patch multi-head attention.

Problem: x [B=4, P=4, N=1024, C=512]; per-patch Wq [P, C, C], Wkv [P, C, 2C];
shared Wproj [C, C], bproj [C]. 8 heads, hd=64.

Sharding: the 16 (b, p) pairs are fully independent; each of the 8 cores
processes 2 pairs (data/expert parallel, no collectives). Wproj/bproj are
replicated.

Per-core pipeline (all matmul operands bf16, accum fp32):
  - x is cast to bf16 then PE-transposed (bf16 transposes are 4x cheaper than
    fp32); transposes land in bitcast views of the idle mm-pool PSUM bank.
  - qT/kT [d, n] computed lazily per head-pair di; scores for head pair di are
    row-tiled (two 64-row K slices of the PE run concurrently), one PSUM slab
    [128, 1024] per (mt, half), double buffered so the scalar engine's exp
    never blocks the next slab's matmuls.
  - exp on the scalar engine straight out of PSUM into SBUF bf16 tiles.
  - v [m, d] has a ones column per 64-wide head block, so attn @ [v|1] also
    yields softmax denominators; out [n, 65] per head accumulates over m in a
    single PSUM bank; reciprocal + per-partition tensor_scalar normalizes.
  - issue order interleaves scores(di) with AV(di-1) one slab group at a time
    so the tensor engine always has ready work while ACT drains exp slabs.
  - o [n, c] is PE-transposed (bf16) and projected with Wproj; bias is added
    by a K=1 ones-row matmul seeding the PSUM accumulation.
"""

import numpy as np

import concourse.bass as bass
import concourse.bacc as bacc
import concourse.mybir as mybir
from concourse.masks import make_identity
from concourse.tile import TileContext

B, P, N, C = 4, 4, 1024, 512
HEADS = 8
HD = C // HEADS  # 64
NT = N // 128  # 8 n-tiles
CCH = C // 128  # 4 c-chunks
F32 = mybir.dt.float32
BF16 = mybir.dt.bfloat16

_CACHE = {}


def _build_kernel():
    nc = bacc.Bacc()
    x = nc.declare_dram_parameter("x", [2, N, C], BF16, False)
    wq = nc.declare_dram_parameter("wq", [2, C, C], BF16, False)
    wkv = nc.declare_dram_parameter("wkv", [2, C, 2 * C], BF16, False)
    wproj = nc.declare_dram_parameter("wproj", [C, C], BF16, False)
    bproj = nc.declare_dram_parameter("bproj", [1, C], BF16, False)
    y = nc.declare_dram_parameter("y", [2, N, C], F32, True)

    with TileContext(nc) as tc:
        with (
            tc.tile_pool(name="consts", bufs=1) as consts,
            tc.tile_pool(name="wpool", bufs=2) as wpool,
            tc.tile_pool(name="xbp", bufs=2) as xbp,
            tc.tile_pool(name="xTp", bufs=2) as xTp,
            tc.tile_pool(name="qkp", bufs=2) as qkp,
            tc.tile_pool(name="vp", bufs=2) as vp,
            tc.tile_pool(name="expp", bufs=2) as expp,
            tc.tile_pool(name="op", bufs=2) as op,
            tc.tile_pool(name="zp", bufs=3) as zp,
            tc.tile_pool(name="smallp", bufs=4) as smallp,
            tc.tile_pool(name="ps_slab", bufs=2, space="PSUM") as ps_slab,
            tc.tile_pool(name="ps_mm", bufs=2, space="PSUM") as ps_mm,
            tc.tile_pool(name="ps_av", bufs=2, space="PSUM") as ps_av,
        ):
            identbf = consts.tile([128, 128], BF16)
            ones_bf = consts.tile([1, 128], BF16)

            # prologue DMAs are issued first (see schedule below); shared
            # proj weights + bias ride the HWDGE queues
            wproj_sb = consts.tile([128, CCH, 512], BF16, name="wproj_sb")
            bp_bf = consts.tile([1, 512], BF16)

            def emit_consts_dma():
                nc.sync.dma_start(
                    out=wproj_sb, in_=wproj.rearrange("(t p) c -> p t c", p=128)
                )
                nc.scalar.dma_start(out=bp_bf, in_=bproj[:, :])

            def mm_bf16_bank():
                # one PSUM bank viewed as [128, 1024] bf16 for PE transposes
                t = ps_mm.tile([128, 512], F32, tag="mm", name="mmbank")
                return t.bitcast(BF16)

            # ---- per-pair state
            S = [
                {
                    "wq": None,
                    "wk": None,
                    "wv": None,
                    "xb": None,
                    "xT": None,
                    "ets": {},
                    "vpad": [None] * NT,
                    "o_sb": [None] * NT,
                    "qk": {},
                }
                for _ in range(2)
            ]
            dma_engs = [nc.sync, nc.scalar, nc.gpsimd]

            def emit_weights(pr):
                st = S[pr]
                for n, (key, src) in enumerate(
                    (
                        ("wq", wq[pr].rearrange("(t p) c -> p t c", p=128)),
                        ("wk", wkv[pr, :, 0:512].rearrange("(t p) c -> p t c", p=128)),
                        ("wv", wkv[pr, :, 512:1024].rearrange("(t p) c -> p t c", p=128)),
                    )
                ):
                    tb = wpool.tile([128, CCH, 512], BF16, tag=key, name=key)
                    nc.gpsimd.dma_start(out=tb, in_=src)
                    st[key] = tb

            def emit_xdma(pr, lo, hi):
                st = S[pr]
                if st["xb"] is None:
                    st["xb"] = xbp.tile([128, NT, 512], BF16, tag="xb", name="xb")
                xsrc = x[pr].rearrange("(t p) c -> p t c", p=128)
                nc.gpsimd.dma_start(out=st["xb"][:, lo:hi, :], in_=xsrc[:, lo:hi, :])

            def emit_xT(pr, nts):
                st = S[pr]
                if st["xT"] is None:
                    st["xT"] = xTp.tile([128, CCH, N], BF16, tag="xT", name="xT")
                for nt in nts:
                    pst = mm_bf16_bank()
                    for ci in range(CCH):
                        nc.tensor.transpose(
                            pst[:, ci * 128 : (ci + 1) * 128],
                            st["xb"][:, nt, ci * 128 : (ci + 1) * 128],
                            identbf,
                        )
                    nc.vector.tensor_copy(
                        st["xT"][:, :, nt * 128 : (nt + 1) * 128],
                        pst[:, 0:512].rearrange("p (c n) -> p c n", n=128),
                    )

            def emit_qkT(pr, di, nfs=(0, 1), which=("q", "k")):
                st = S[pr]
                if di not in st["qk"]:
                    st["qk"][di] = (
                        qkp.tile([128, N], BF16, tag=f"q{di % 2}", name="qt"),
                        qkp.tile([128, N], BF16, tag=f"k{di % 2}", name="kt"),
                    )
                qt, kt = st["qk"][di]
                dcols = slice(di * 128, (di + 1) * 128)
                tgts = {"q": (qt, st["wq"]), "k": (kt, st["wk"])}
                for nf in nfs:
                    ncols = slice(nf * 512, (nf + 1) * 512)
                    for dst, wsb in [tgts[w] for w in which]:
                        ps = ps_mm.tile([128, 512], F32, tag="mm", name="mmqk")
                        for ci in range(CCH):
                            nc.tensor.matmul(
                                ps,
                                wsb[:, ci, dcols],
                                st["xT"][:, ci, ncols],
                                start=(ci == 0),
                                stop=(ci == CCH - 1),
                            )
                        nc.vector.tensor_copy(dst[:, ncols], ps)
                return qt, kt

            def emit_v(pr, mt):
                st = S[pr]
                ps = ps_mm.tile([128, 512], F32, tag="mm", name="mmv")
                for ci in range(CCH):
                    nc.tensor.matmul(
                        ps,
                        st["xT"][:, ci, mt * 128 : (mt + 1) * 128],
                        st["wv"][:, ci, :],
                        start=(ci == 0),
                        stop=(ci == CCH - 1),
                    )
                vv = vp.tile([128, HEADS * 65], BF16, tag=f"v{mt}", name=f"v{mt}")
                vr = vv.rearrange("p (h w) -> p h w", w=65)
                nc.vector.memset(vr[:, :, 64:65], 1.0)
                nc.vector.tensor_copy(
                    vr[:, :, 0:64], ps.rearrange("p (h w) -> p h w", w=64)
                )
                st["vpad"][mt] = vv

            def emit_scores(pr, di, mt):
                st = S[pr]
                qt, kt = st["qk"][di]
                slabs = [
                    ps_slab.tile([128, 1024], F32, tag="slab", name="slab")
                    for _ in range(2)
                ]
                for nf in range(2):
                    for half in range(2):
                        prow = slice(half * 64, (half + 1) * 64)
                        nc.tensor.matmul(
                            slabs[half][:, nf * 512 : (nf + 1) * 512],
                            kt[prow, mt * 128 : (mt + 1) * 128],
                            qt[prow, nf * 512 : (nf + 1) * 512],
                            start=True,
                            stop=True,
                        )
                for half in range(2):
                    et = expp.tile(
                        [128, 1024], BF16, tag=f"e{mt}_{half}", name="et", bufs=2
                    )
                    nc.scalar.activation(
                        et, slabs[half], mybir.ActivationFunctionType.Exp, scale=0.125
                    )
                    st["ets"][(di, mt, half)] = et

            def emit_av(pr, di, nt):
                st = S[pr]
                if st["o_sb"][nt] is None:
                    st["o_sb"][nt] = op.tile([128, C], BF16, tag=f"o{nt}", name="osb")
                av = ps_av.tile([128, 130], F32, tag="av", name="av")
                for half in range(2):
                    h = 2 * di + half
                    for mt in range(NT):
                        nc.tensor.matmul(
                            av[:, half * 65 : (half + 1) * 65],
                            st["ets"][(di, mt, half)][:, nt * 128 : (nt + 1) * 128],
                            st["vpad"][mt][:, h * 65 : (h + 1) * 65],
                            start=(mt == 0),
                            stop=(mt == NT - 1),
                        )
                rc = smallp.tile([128, 2], F32, tag="rc", name="rc")
                nc.vector.reciprocal(rc, av[:, 64:130:65])
                for half in range(2):
                    h = 2 * di + half
                    nc.vector.tensor_scalar_mul(
                        st["o_sb"][nt][:, h * 64 : (h + 1) * 64],
                        av[:, half * 65 : half * 65 + 64],
                        rc[:, half : half + 1],
                    )

            def emit_proj(pr, nt, on_act=False):
                st = S[pr]
                pst = mm_bf16_bank()
                for ci in range(CCH):
                    nc.tensor.transpose(
                        pst[:, ci * 128 : (ci + 1) * 128],
                        st["o_sb"][nt][:, ci * 128 : (ci + 1) * 128],
                        identbf,
                    )
                oTn = op.tile([128, CCH, 128], BF16, tag="oT", name="oT")
                nc.vector.tensor_copy(
                    oTn, pst[:, 0:512].rearrange("p (c n) -> p c n", n=128)
                )
                zps = ps_mm.tile([128, 512], F32, tag="mm", name="mmz")
                nc.tensor.matmul(
                    zps, ones_bf[0:1, :], bp_bf[0:1, :], start=True, stop=False
                )
                for ci in range(CCH):
                    nc.tensor.matmul(
                        zps,
                        oTn[:, ci, :],
                        wproj_sb[:, ci, :],
                        start=False,
                        stop=(ci == CCH - 1),
                    )
                zsb = zp.tile([128, 512], F32, tag="z", name="z")
                if on_act:
                    nc.scalar.copy(zsb, zps)
                else:
                    nc.vector.tensor_copy(zsb, zps)
                nc.gpsimd.dma_start(out=y[pr, nt * 128 : (nt + 1) * 128, :], in_=zsb)
                st["o_sb"][nt] = None

            # ---- software-pipelined schedule across both pairs.
            # The scalar engine (exp) paces the scores phases; AV/proj/loads
            # of the other pair fill the tensor/DVE/DMA engines in between.
            emit_xdma(0, 0, 4)
            emit_xdma(0, 4, NT)
            emit_weights(0)
            make_identity(nc, identbf)
            nc.vector.memset(ones_bf, 1.0)
            emit_consts_dma()
            emit_xT(0, range(0, 4))
            emit_qkT(0, 0, nfs=(0,), which=("q",))
            emit_xT(0, range(4, NT))
            emit_qkT(0, 0, nfs=(1,), which=("q",))
            emit_qkT(0, 0, nfs=(0,), which=("k",))
            for mt in range(4):
                emit_scores(0, 0, mt)
                emit_v(0, mt)
            emit_qkT(0, 0, nfs=(1,), which=("k",))
            for mt in range(4, NT):
                emit_scores(0, 0, mt)
                emit_v(0, mt)
            emit_qkT(0, 1)
            for mt in range(NT):
                emit_av(0, 0, mt)
                emit_scores(0, 1, mt)
            emit_qkT(0, 2)
            emit_weights(1)
            emit_xdma(1, 0, NT)
            for mt in range(NT):
                emit_av(0, 1, mt)
                emit_xT(1, [mt])
                emit_scores(0, 2, mt)
            emit_qkT(0, 3)
            for mt in range(NT):
                emit_av(0, 2, mt)
                emit_scores(0, 3, mt)
            emit_qkT(1, 0)
            for mt in range(NT):
                emit_av(0, 3, mt)
                emit_v(1, mt)
                emit_scores(1, 0, mt)
            emit_qkT(1, 1)
            for mt in range(NT):
                emit_av(1, 0, mt)
                if mt % 2 == 0:
                    emit_proj(0, mt // 2)
                emit_scores(1, 1, mt)
            emit_qkT(1, 2)
            for mt in range(NT):
                emit_av(1, 1, mt)
                if mt % 2 == 0:
                    emit_proj(0, 4 + mt // 2)
                emit_scores(1, 2, mt)
            emit_qkT(1, 3)
            for mt in range(NT):
                emit_av(1, 2, mt)
                emit_scores(1, 3, mt)
            for nt in range(NT):
                emit_av(1, 3, nt)
                emit_proj(1, nt, on_act=True)
    return nc


def _get_nc():
    if "nc" not in _CACHE:
        nc = _build_kernel()
        nc.compile()
        _CACHE["nc"] = nc
    return _CACHE["nc"]


def kernel(**inputs) -> np.ndarray:
    import ml_dtypes

    from concourse.bass_utils import run_bass_kernel_spmd

    BF = ml_dtypes.bfloat16
    x = np.ascontiguousarray(np.asarray(inputs["x"], dtype=np.float32).astype(BF))
    Wq = np.ascontiguousarray(np.asarray(inputs["Wq"], dtype=np.float32).astype(BF))
    Wkv = np.ascontiguousarray(np.asarray(inputs["Wkv"], dtype=np.float32).astype(BF))
    Wproj = np.ascontiguousarray(
        np.asarray(inputs["Wproj"], dtype=np.float32).astype(BF)
    )
    bproj = np.ascontiguousarray(
        np.asarray(inputs["bproj"], dtype=np.float32).reshape(1, C).astype(BF)
    )

    nc = _get_nc()
    xr = x.reshape(B * P, N, C)
    in_maps = []
    for core in range(8):
        p0 = (2 * core) % P
        in_maps.append(
            {
                "x": np.ascontiguousarray(xr[2 * core : 2 * core + 2]),
                "wq": np.ascontiguousarray(Wq[p0 : p0 + 2]),
                "wkv": np.ascontiguousarray(Wkv[p0 : p0 + 2]),
                "wproj": Wproj,
                "bproj": bproj,
            }
        )
    res = run_bass_kernel_spmd(nc, in_maps, list(range(8))).results
    out = np.concatenate([r["y"] for r in res], axis=0).reshape(B, P, N, C)
    return out.astype(np.float32)
